# revision 34
# baseline (speedup 1.0000x reference)
# MoE-routing kernel for Trainium2: out[b] = x[b] @ weight[y[b]] + bias[y[b]]
# x: [1024, 64, 1152] f32, y: [1024] int64, weight: [1000, 1152, 128] f32,
# bias: [1000, 128] f32 -> out: [1024, 64, 128] f32.
#
# y has ~646 distinct classes among 1024 samples, so the dominant weight
# traffic deduplicates: samples are grouped by class into "jobs", jobs are
# dealt to 8 cores so every core carries exactly 128 samples and a
# near-identical job-size profile. SPMD needs one instruction stream, so the
# per-rank job sizes are the max envelope over cores (a few ghost slots pad
# the difference; their x is zeros and their output is discarded).
# Per job the weight k-tile [128,128] is the stationary matmul operand and
# all the job's samples stream through as one moving operand [128, P*64],
# accumulating over the 9 k-tiles in PSUM. Weight loads drop from 128 to
# ~81 per core: ~45 MB/core of HBM traffic vs 59 MB for the undeduplicated
# version. x DMA runs on the Sync engine, weights on GpSimd, outputs on
# Scalar so the streams don't serialize.
#
# DMA descriptor size dominates achieved bandwidth: the 16 shared DMA
# engines run ~25 GB/s each on 2304B descriptors but only ~18 GB/s on 9KB
# ones. x and w therefore live chunk-major in DRAM ([chunk, 128, 1152elem])
# so every descriptor is a 2304B partition-line.

import numpy as np
from collections import defaultdict, Counter

B, N, HIDDEN = 1024, 64, 1152
NUM_CLASSES = 1000
OUT_DIM = 128
KT = HIDDEN // 128  # 9 k-tiles
NCORES = 8
S = B // NCORES      # 128 samples per core
PMAX = 8             # max samples per job (PSUM bank: 8*64 f32 = 2KB)

_cache = {}


def _schedule(y):
    """Deal class groups to cores; return per-core job lists and the uniform
    job-size profile P (ascending)."""
    cnt = defaultdict(int)
    for v in y:
        cnt[int(v)] += 1
    # split groups larger than PMAX, sort descending for LPT dealing
    groups = []
    for c, k in cnt.items():
        while k > PMAX:
            groups.append([c, PMAX])
            k -= PMAX
        if k:
            groups.append([c, k])
    groups.sort(key=lambda g: (-g[1], g[0]))
    cores = [[] for _ in range(NCORES)]
    sums = [0] * NCORES
    for c, k in groups:
        m = min(range(NCORES), key=lambda m: (sums[m], len(cores[m])))
        cores[m].append([c, k])
        sums[m] += k
    # rebalance to exactly S samples per core
    for _ in range(B):
        hi = max(range(NCORES), key=lambda m: sums[m])
        lo = min(range(NCORES), key=lambda m: sums[m])
        if sums[hi] <= S and sums[lo] >= S:
            break
        g1 = next((g for g in cores[hi] if g[1] == 1), None)
        if g1 is not None:
            cores[hi].remove(g1)
            cores[lo].append(g1)
        else:
            g = max(cores[hi], key=lambda g: g[1])
            g[1] -= 1
            cores[lo].append([g[0], 1])
        sums[hi] -= 1
        sums[lo] += 1
    assert all(s == S for s in sums)
    # ascending job size: tiny jobs first => cheap pipeline-fill groups
    for cr in cores:
        cr.sort(key=lambda g: (g[1], g[0]))
    U = max(len(cr) for cr in cores)
    # right-align job lists so ranks line up size-ascending across cores
    for cr in cores:
        while len(cr) < U:
            cr.insert(0, [-1, 0])  # padding job: no class, no samples
    P = [max(max(cr[j][1] for cr in cores), 1) for j in range(U)]
    if sum(P) % 2:  # x DMA chunks cover 2 slots; keep V even
        P.append(1)
        for cr in cores:
            cr.append([-1, 0])
    # odd-size jobs first (their count is even, so they pair to even slot
    # boundaries); ascending within each class so the pipeline ramps cheap
    order = sorted(range(len(P)), key=lambda j: (P[j] % 2 == 0, P[j]))
    P = [P[j] for j in order]
    for m in range(NCORES):
        cores[m] = [cores[m][j] for j in order]
    return cores, P


def _make_groups(P):
    """Split jobs into DMA groups: (j0, j1, v0, v1) with a slot-count ramp.
    Group slot boundaries stay even so x chunks (2 slots) never split."""
    off = np.concatenate([[0], np.cumsum(P)]).astype(int)
    budgets = [2, 2, 4]
    groups = []
    j = 0
    U = len(P)
    V = int(off[-1])
    while j < U:
        b = budgets[len(groups)] if len(groups) < len(budgets) else 12
        j1 = j
        slots = 0
        while j1 < U and (slots < b or off[j1] % 2) and (j1 - j) < 10:
            slots += P[j1]
            j1 += 1
        while j1 < U and off[j1] % 2:
            j1 += 1
        groups.append((j, j1, int(off[j]), int(off[j1])))
        j = j1
    assert all(v0 % 2 == 0 and v1 % 2 == 0 for _, _, v0, v1 in groups)
    return groups, off


def _build_nc(P):
    import concourse.bass as bass
    import concourse.mybir as mybir
    from concourse.tile import TileContext

    groups, off = _make_groups(P)
    U = len(P)
    V = int(sum(P))

    nc = bass.Bass()
    f32 = mybir.dt.float32
    bf16 = mybir.dt.bfloat16
    # chunk-major DRAM layouts: one 2304B descriptor per (chunk, partition)
    Xd = nc.declare_dram_parameter("xin", [V // 2, 128, 1152], bf16, isOutput=False)
    Wd = nc.declare_dram_parameter("win", [U, 128, 1152], bf16, isOutput=False)
    Od = nc.declare_dram_parameter("o", [128, V * 64], bf16, isOutput=True)

    with TileContext(nc) as tc:
        with (
            tc.tile_pool(name="xp", bufs=4) as xp,
            tc.tile_pool(name="wp", bufs=4) as wp,
            tc.tile_pool(name="op", bufs=4) as op,
            tc.tile_pool(name="pp", bufs=8, space="PSUM") as pp,
        ):
            # one dedicated trigger queue per stream: a ring-full lock-wait on
            # one stream then never delays the other streams' triggers
            for gi, (j0, j1, v0, v1) in enumerate(groups):
                xt = xp.tile([128, (v1 - v0) * 576], bf16, tag="xt")
                nc.sync.dma_start(
                    out=xt, in_=Xd[v0 // 2 : v1 // 2].rearrange("c p e -> p c e")
                )
                wt = wp.tile([128, (j1 - j0) * 1152], bf16, tag="wt")
                nc.gpsimd.dma_start(
                    out=wt, in_=Wd[j0:j1].rearrange("j p e -> p j e")
                )
                ot = op.tile([128, (v1 - v0) * 64], bf16, tag="ot")
                for j in range(j0, j1):
                    pj = P[j]
                    xbase = (off[j] - v0) * 576  # job block: [KT, pj, 64]
                    ps = pp.tile([128, pj * 64], f32)
                    for k in range(KT):
                        nc.tensor.matmul(
                            ps,
                            wt[:, (j - j0) * 1152 + k * 128 : (j - j0) * 1152 + (k + 1) * 128],
                            xt[:, xbase + k * pj * 64 : xbase + (k + 1) * pj * 64],
                            start=(k == 0),
                            stop=(k == KT - 1),
                        )
                    obase = (off[j] - v0) * 64
                    nc.vector.tensor_copy(ot[:, obase : obase + pj * 64], ps)
                nc.scalar.dma_start(out=Od[:, v0 * 64 : v1 * 64], in_=ot)

    _split_excess_waits(nc)
    nc.finalize()
    _split_excess_waits(nc)
    return nc


def _split_excess_waits(nc, max_waits=1):
    # walrus codegen rejects instructions with >max sync waits; Tile's tail
    # drain can carry several. Hoist the excess onto preceding no-ops.
    import concourse.mybir as mybir

    for f in nc.m.functions:
        for b in f.blocks:
            i = 0
            while i < len(b.instructions):
                inst = b.instructions[i]
                si = inst.sync_info
                if si is not None and len(si.on_wait) > max_waits:
                    excess = list(si.on_wait[:-max_waits])
                    si.on_wait = list(si.on_wait[-max_waits:])
                    for w in excess:
                        nop = mybir.InstNoOp(
                            name=nc.get_next_instruction_name(),
                            engine=inst.engine,
                            sync_info=mybir.SyncInfo(on_wait=[w], on_update=[]),
                            bass_nofuse=True,
                        )
                        nc.register_instruction(nop)
                        b.instructions.insert(i, nop)
                        i += 1
                i += 1


def _prep_inputs(x, y, weight, cores, P):
    import ml_dtypes

    bf16 = ml_dtypes.bfloat16
    U = len(P)
    V = int(sum(P))
    off = np.concatenate([[0], np.cumsum(P)]).astype(int)

    x = np.ascontiguousarray(x, dtype=np.float32)
    weight = np.ascontiguousarray(weight, dtype=np.float32)
    # x[b, n, k*128+p] -> Xh[b, p, k, n]
    Xh = np.ascontiguousarray(x.reshape(B, N, KT, 128).transpose(0, 3, 2, 1)).astype(bf16)
    # weight[c, k*128+p, o] -> Wp[c, p, k, o] for the used classes only
    used = sorted({c for cr in cores for c, k in cr if c >= 0})
    cidx = {c: i for i, c in enumerate(used)}
    Wsel = weight[np.asarray(used, dtype=np.int64)]
    Wp = np.ascontiguousarray(
        Wsel.reshape(len(used), KT, 128, OUT_DIM).transpose(0, 2, 1, 3)
    ).reshape(len(used), 128, KT * OUT_DIM).astype(bf16)

    by_class = defaultdict(list)
    for i, c in enumerate(np.asarray(y).astype(np.int64)):
        by_class[int(c)].append(i)

    Xin = np.zeros((NCORES, 128, V * 576), dtype=bf16)
    Win = np.zeros((NCORES, U, 128, 1152), dtype=bf16)
    slotmap = np.full((NCORES, V), -1, dtype=np.int64)
    for m in range(NCORES):
        for j, (c, k) in enumerate(cores[m]):
            if k == 0:
                continue
            bs = by_class[c][:k]
            del by_class[c][:k]
            pj = P[j]
            # job block [128, KT, pj, 64]; real samples fill the first k lanes
            blk = Xin[m, :, off[j] * 576 : off[j + 1] * 576].reshape(128, KT, pj, 64)
            blk[:, :, :k, :] = Xh[bs].transpose(1, 2, 0, 3)
            Win[m, j] = Wp[cidx[c]]
            slotmap[m, off[j] : off[j] + k] = bs
    # chunk-major x: [V/2, 128, 1152elem]
    Xin = np.ascontiguousarray(Xin.reshape(NCORES, 128, V // 2, 1152).transpose(0, 2, 1, 3))
    return Xin, Win, slotmap, V


def kernel(x, y, weight, bias):
    from concourse.bass_utils import run_bass_kernel_spmd

    yi = np.asarray(y).astype(np.int64)
    cores, P = _schedule(yi)
    key = tuple(P)
    if _cache.get("key") != key:
        _cache["nc"] = _build_nc(P)
        _cache["key"] = key
    nc = _cache["nc"]

    Xin, Win, slotmap, V = _prep_inputs(x, y, weight, cores, P)
    in_maps = [{"xin": Xin[m], "win": Win[m]} for m in range(NCORES)]
    res = run_bass_kernel_spmd(nc, in_maps, list(range(NCORES)), **_cache.get("runkw", {}))
    _cache["last_result"] = res

    out = np.empty((B, N, OUT_DIM), dtype=np.float32)
    for m in range(NCORES):
        Oc = np.asarray(res.results[m]["o"], dtype=np.float32)
        Oc = Oc.reshape(128, V, 64).transpose(1, 2, 0)  # [slot, n, o]
        real = slotmap[m] >= 0
        out[slotmap[m][real]] = Oc[real]
    out += np.asarray(bias, dtype=np.float32)[yi][:, None, :]
    return out


# revision 35
# speedup vs baseline: 1.0917x; 1.0917x over previous
# MoE-routing kernel for Trainium2: out[b] = x[b] @ weight[y[b]] + bias[y[b]]
# x: [1024, 64, 1152] f32, y: [1024] int64, weight: [1000, 1152, 128] f32,
# bias: [1000, 128] f32 -> out: [1024, 64, 128] f32.
#
# y has ~646 distinct classes among 1024 samples, so the dominant weight
# traffic deduplicates: samples are grouped by class into "jobs", jobs are
# dealt to 8 cores so every core carries exactly 128 samples and a
# near-identical job-size profile. SPMD needs one instruction stream, so the
# per-rank job sizes are the max envelope over cores (a few ghost slots pad
# the difference; their x is zeros and their output is discarded).
# Per job the weight k-tile [128,128] is the stationary matmul operand and
# all the job's samples stream through as one moving operand [128, P*64],
# accumulating over the 9 k-tiles in PSUM. Weight loads drop from 128 to
# ~81 per core: ~45 MB/core of HBM traffic vs 59 MB for the undeduplicated
# version. x DMA runs on the Sync engine, weights on GpSimd, outputs on
# Scalar so the streams don't serialize.
#
# DMA descriptor size dominates achieved bandwidth: the 16 shared DMA
# engines run ~25 GB/s each on 2304B descriptors but only ~18 GB/s on 9KB
# ones. x and w therefore live chunk-major in DRAM ([chunk, 128, 1152elem])
# so every descriptor is a 2304B partition-line.

import numpy as np
from collections import defaultdict, Counter

B, N, HIDDEN = 1024, 64, 1152
NUM_CLASSES = 1000
OUT_DIM = 128
KT = HIDDEN // 128  # 9 k-tiles
NCORES = 8
S = B // NCORES      # 128 samples per core
PMAX = 8             # max samples per job (PSUM bank: 8*64 f32 = 2KB)

_cache = {}


def _schedule(y):
    """Deal class groups to cores; return per-core job lists and the uniform
    job-size profile P (ascending)."""
    cnt = defaultdict(int)
    for v in y:
        cnt[int(v)] += 1
    # split groups larger than PMAX, sort descending for LPT dealing
    groups = []
    for c, k in cnt.items():
        while k > PMAX:
            groups.append([c, PMAX])
            k -= PMAX
        if k:
            groups.append([c, k])
    groups.sort(key=lambda g: (-g[1], g[0]))
    cores = [[] for _ in range(NCORES)]
    sums = [0] * NCORES
    for c, k in groups:
        m = min(range(NCORES), key=lambda m: (sums[m], len(cores[m])))
        cores[m].append([c, k])
        sums[m] += k
    # rebalance to exactly S samples per core
    for _ in range(B):
        hi = max(range(NCORES), key=lambda m: sums[m])
        lo = min(range(NCORES), key=lambda m: sums[m])
        if sums[hi] <= S and sums[lo] >= S:
            break
        g1 = next((g for g in cores[hi] if g[1] == 1), None)
        if g1 is not None:
            cores[hi].remove(g1)
            cores[lo].append(g1)
        else:
            g = max(cores[hi], key=lambda g: g[1])
            g[1] -= 1
            cores[lo].append([g[0], 1])
        sums[hi] -= 1
        sums[lo] += 1
    assert all(s == S for s in sums)
    # ascending job size: tiny jobs first => cheap pipeline-fill groups
    for cr in cores:
        cr.sort(key=lambda g: (g[1], g[0]))
    U = max(len(cr) for cr in cores)
    # right-align job lists so ranks line up size-ascending across cores
    for cr in cores:
        while len(cr) < U:
            cr.insert(0, [-1, 0])  # padding job: no class, no samples
    P = [max(max(cr[j][1] for cr in cores), 1) for j in range(U)]
    if sum(P) % 2:  # x DMA chunks cover 2 slots; keep V even
        P.append(1)
        for cr in cores:
            cr.append([-1, 0])
    # odd-size jobs first (their count is even, so they pair to even slot
    # boundaries); ascending within each class so the pipeline ramps cheap
    order = sorted(range(len(P)), key=lambda j: (P[j] % 2 == 0, P[j]))
    P = [P[j] for j in order]
    for m in range(NCORES):
        cores[m] = [cores[m][j] for j in order]
    return cores, P


def _make_groups(P):
    """Split jobs into DMA groups: (j0, j1, v0, v1) with a slot-count ramp.
    Group slot boundaries stay even so x chunks (2 slots) never split."""
    off = np.concatenate([[0], np.cumsum(P)]).astype(int)
    budgets = [2, 2, 4]
    groups = []
    j = 0
    U = len(P)
    V = int(off[-1])
    while j < U:
        b = budgets[len(groups)] if len(groups) < len(budgets) else 8
        j1 = j
        slots = 0
        while j1 < U and (slots < b or off[j1] % 2) and (j1 - j) < 10:
            slots += P[j1]
            j1 += 1
        while j1 < U and off[j1] % 2:
            j1 += 1
        groups.append((j, j1, int(off[j]), int(off[j1])))
        j = j1
    assert all(v0 % 2 == 0 and v1 % 2 == 0 for _, _, v0, v1 in groups)
    return groups, off


def _build_nc(P):
    import concourse.bass as bass
    import concourse.mybir as mybir
    from concourse.tile import TileContext

    groups, off = _make_groups(P)
    U = len(P)
    V = int(sum(P))

    nc = bass.Bass()
    f32 = mybir.dt.float32
    bf16 = mybir.dt.bfloat16
    # chunk-major DRAM layouts: one 2304B descriptor per (chunk, partition)
    Xd = nc.declare_dram_parameter("xin", [V // 2, 128, 1152], bf16, isOutput=False)
    Wd = nc.declare_dram_parameter("win", [U, 128, 1152], bf16, isOutput=False)
    Od = nc.declare_dram_parameter("o", [128, V * 64], bf16, isOutput=True)

    with TileContext(nc) as tc:
        with (
            tc.tile_pool(name="xp", bufs=4) as xp,
            tc.tile_pool(name="wp", bufs=4) as wp,
            tc.tile_pool(name="op", bufs=4) as op,
            tc.tile_pool(name="pp", bufs=8, space="PSUM") as pp,
        ):
            # one dedicated trigger queue per stream: a ring-full lock-wait on
            # one stream then never delays the other streams' triggers
            for gi, (j0, j1, v0, v1) in enumerate(groups):
                xt = xp.tile([128, (v1 - v0) * 576], bf16, tag="xt")
                nc.sync.dma_start(
                    out=xt, in_=Xd[v0 // 2 : v1 // 2].rearrange("c p e -> p c e")
                )
                wt = wp.tile([128, (j1 - j0) * 1152], bf16, tag="wt")
                nc.gpsimd.dma_start(
                    out=wt, in_=Wd[j0:j1].rearrange("j p e -> p j e")
                )
                ot = op.tile([128, (v1 - v0) * 64], bf16, tag="ot")
                for j in range(j0, j1):
                    pj = P[j]
                    xbase = (off[j] - v0) * 576  # job block: [KT, pj, 64]
                    ps = pp.tile([128, pj * 64], f32)
                    for k in range(KT):
                        nc.tensor.matmul(
                            ps,
                            wt[:, (j - j0) * 1152 + k * 128 : (j - j0) * 1152 + (k + 1) * 128],
                            xt[:, xbase + k * pj * 64 : xbase + (k + 1) * pj * 64],
                            start=(k == 0),
                            stop=(k == KT - 1),
                        )
                    obase = (off[j] - v0) * 64
                    nc.vector.tensor_copy(ot[:, obase : obase + pj * 64], ps)
                nc.scalar.dma_start(out=Od[:, v0 * 64 : v1 * 64], in_=ot)

    _split_excess_waits(nc)
    nc.finalize()
    _split_excess_waits(nc)
    return nc


def _split_excess_waits(nc, max_waits=1):
    # walrus codegen rejects instructions with >max sync waits; Tile's tail
    # drain can carry several. Hoist the excess onto preceding no-ops.
    import concourse.mybir as mybir

    for f in nc.m.functions:
        for b in f.blocks:
            i = 0
            while i < len(b.instructions):
                inst = b.instructions[i]
                si = inst.sync_info
                if si is not None and len(si.on_wait) > max_waits:
                    excess = list(si.on_wait[:-max_waits])
                    si.on_wait = list(si.on_wait[-max_waits:])
                    for w in excess:
                        nop = mybir.InstNoOp(
                            name=nc.get_next_instruction_name(),
                            engine=inst.engine,
                            sync_info=mybir.SyncInfo(on_wait=[w], on_update=[]),
                            bass_nofuse=True,
                        )
                        nc.register_instruction(nop)
                        b.instructions.insert(i, nop)
                        i += 1
                i += 1


def _prep_inputs(x, y, weight, cores, P):
    import ml_dtypes

    bf16 = ml_dtypes.bfloat16
    U = len(P)
    V = int(sum(P))
    off = np.concatenate([[0], np.cumsum(P)]).astype(int)

    x = np.ascontiguousarray(x, dtype=np.float32)
    weight = np.ascontiguousarray(weight, dtype=np.float32)
    # x[b, n, k*128+p] -> Xh[b, p, k, n]
    Xh = np.ascontiguousarray(x.reshape(B, N, KT, 128).transpose(0, 3, 2, 1)).astype(bf16)
    # weight[c, k*128+p, o] -> Wp[c, p, k, o] for the used classes only
    used = sorted({c for cr in cores for c, k in cr if c >= 0})
    cidx = {c: i for i, c in enumerate(used)}
    Wsel = weight[np.asarray(used, dtype=np.int64)]
    Wp = np.ascontiguousarray(
        Wsel.reshape(len(used), KT, 128, OUT_DIM).transpose(0, 2, 1, 3)
    ).reshape(len(used), 128, KT * OUT_DIM).astype(bf16)

    by_class = defaultdict(list)
    for i, c in enumerate(np.asarray(y).astype(np.int64)):
        by_class[int(c)].append(i)

    Xin = np.zeros((NCORES, 128, V * 576), dtype=bf16)
    Win = np.zeros((NCORES, U, 128, 1152), dtype=bf16)
    slotmap = np.full((NCORES, V), -1, dtype=np.int64)
    for m in range(NCORES):
        for j, (c, k) in enumerate(cores[m]):
            if k == 0:
                continue
            bs = by_class[c][:k]
            del by_class[c][:k]
            pj = P[j]
            # job block [128, KT, pj, 64]; real samples fill the first k lanes
            blk = Xin[m, :, off[j] * 576 : off[j + 1] * 576].reshape(128, KT, pj, 64)
            blk[:, :, :k, :] = Xh[bs].transpose(1, 2, 0, 3)
            Win[m, j] = Wp[cidx[c]]
            slotmap[m, off[j] : off[j] + k] = bs
    # chunk-major x: [V/2, 128, 1152elem]
    Xin = np.ascontiguousarray(Xin.reshape(NCORES, 128, V // 2, 1152).transpose(0, 2, 1, 3))
    return Xin, Win, slotmap, V


def kernel(x, y, weight, bias):
    from concourse.bass_utils import run_bass_kernel_spmd

    yi = np.asarray(y).astype(np.int64)
    cores, P = _schedule(yi)
    key = tuple(P)
    if _cache.get("key") != key:
        _cache["nc"] = _build_nc(P)
        _cache["key"] = key
    nc = _cache["nc"]

    Xin, Win, slotmap, V = _prep_inputs(x, y, weight, cores, P)
    in_maps = [{"xin": Xin[m], "win": Win[m]} for m in range(NCORES)]
    res = run_bass_kernel_spmd(nc, in_maps, list(range(NCORES)), **_cache.get("runkw", {}))
    _cache["last_result"] = res

    out = np.empty((B, N, OUT_DIM), dtype=np.float32)
    for m in range(NCORES):
        Oc = np.asarray(res.results[m]["o"], dtype=np.float32)
        Oc = Oc.reshape(128, V, 64).transpose(1, 2, 0)  # [slot, n, o]
        real = slotmap[m] >= 0
        out[slotmap[m][real]] = Oc[real]
    out += np.asarray(bias, dtype=np.float32)[yi][:, None, :]
    return out


# revision 37
# speedup vs baseline: 1.1470x; 1.0507x over previous
# MoE-routing kernel for Trainium2: out[b] = x[b] @ weight[y[b]] + bias[y[b]]
# x: [1024, 64, 1152] f32, y: [1024] int64, weight: [1000, 1152, 128] f32,
# bias: [1000, 128] f32 -> out: [1024, 64, 128] f32.
#
# y has ~646 distinct classes among 1024 samples, so the dominant weight
# traffic deduplicates: samples are grouped by class into "jobs", jobs are
# dealt to 8 cores so every core carries exactly 128 samples and a
# near-identical job-size profile. SPMD needs one instruction stream, so the
# per-rank job sizes are the max envelope over cores (a few ghost slots pad
# the difference; their x is zeros and their output is discarded).
# Per job the weight k-tile [128,128] is the stationary matmul operand and
# all the job's samples stream through as one moving operand [128, P*64],
# accumulating over the 9 k-tiles in PSUM. Weight loads drop from 128 to
# ~81 per core: ~45 MB/core of HBM traffic vs 59 MB for the undeduplicated
# version. x DMA runs on the Sync engine, weights on GpSimd, outputs on
# Scalar so the streams don't serialize.
#
# DMA descriptor size dominates achieved bandwidth: the 16 shared DMA
# engines run ~25 GB/s each on 2304B descriptors but only ~18 GB/s on 9KB
# ones. x and w therefore live chunk-major in DRAM ([chunk, 128, 1152elem])
# so every descriptor is a 2304B partition-line.

import numpy as np
from collections import defaultdict

B, N, HIDDEN = 1024, 64, 1152
NUM_CLASSES = 1000
OUT_DIM = 128
KT = HIDDEN // 128  # 9 k-tiles
NCORES = 8
S = B // NCORES      # 128 samples per core
PMAX = 8             # max samples per job (PSUM bank: 8*64 f32 = 2KB)

_cache = {}


def _schedule(y):
    """Deal class groups to cores; return per-core job lists and the uniform
    job-size profile P (odd sizes first, then evens, ascending within each)."""
    cnt = defaultdict(int)
    for v in y:
        cnt[int(v)] += 1
    # split groups larger than PMAX, sort descending for LPT dealing
    groups = []
    for c, k in cnt.items():
        while k > PMAX:
            groups.append([c, PMAX])
            k -= PMAX
        if k:
            groups.append([c, k])
    groups.sort(key=lambda g: (-g[1], g[0]))
    cores = [[] for _ in range(NCORES)]
    sums = [0] * NCORES
    for c, k in groups:
        m = min(range(NCORES), key=lambda m: (sums[m], len(cores[m])))
        cores[m].append([c, k])
        sums[m] += k
    # rebalance to exactly S samples per core
    for _ in range(B):
        hi = max(range(NCORES), key=lambda m: sums[m])
        lo = min(range(NCORES), key=lambda m: sums[m])
        if sums[hi] <= S and sums[lo] >= S:
            break
        g1 = next((g for g in cores[hi] if g[1] == 1), None)
        if g1 is not None:
            cores[hi].remove(g1)
            cores[lo].append(g1)
        else:
            g = max(cores[hi], key=lambda g: g[1])
            g[1] -= 1
            cores[lo].append([g[0], 1])
        sums[hi] -= 1
        sums[lo] += 1
    assert all(s == S for s in sums)
    # ascending job size: tiny jobs first => cheap pipeline-fill groups
    for cr in cores:
        cr.sort(key=lambda g: (g[1], g[0]))
    U = max(len(cr) for cr in cores)
    # right-align job lists so ranks line up size-ascending across cores
    for cr in cores:
        while len(cr) < U:
            cr.insert(0, [-1, 0])  # padding job: no class, no samples
    P = [max(max(cr[j][1] for cr in cores), 1) for j in range(U)]
    if sum(P) % 2:  # x DMA chunks cover 2 slots; keep V even
        P.append(1)
        for cr in cores:
            cr.append([-1, 0])
    # odd-size jobs first (their count is even, so they pair to even slot
    # boundaries); ascending within each class so the pipeline ramps cheap
    order = sorted(range(len(P)), key=lambda j: (P[j] % 2 == 0, P[j]))
    P = [P[j] for j in order]
    for m in range(NCORES):
        cores[m] = [cores[m][j] for j in order]
    return cores, P


def _make_groups(P):
    """Split jobs into DMA groups: (j0, j1, v0, v1) with a slot-count ramp.
    Group slot boundaries stay even so x chunks (2 slots) never split."""
    off = np.concatenate([[0], np.cumsum(P)]).astype(int)
    budgets = [2, 2, 4]
    groups = []
    j = 0
    U = len(P)
    V = int(off[-1])
    while j < U:
        b = budgets[len(groups)] if len(groups) < len(budgets) else 8
        j1 = j
        slots = 0
        while j1 < U and (slots < b or off[j1] % 2) and (j1 - j) < 10:
            slots += P[j1]
            j1 += 1
        while j1 < U and off[j1] % 2:
            j1 += 1
        groups.append((j, j1, int(off[j]), int(off[j1])))
        j = j1
    assert all(v0 % 2 == 0 and v1 % 2 == 0 for _, _, v0, v1 in groups)
    return groups, off


def _build_nc(P):
    import concourse.bass as bass
    import concourse.mybir as mybir
    from concourse.tile import TileContext

    groups, off = _make_groups(P)
    U = len(P)
    V = int(sum(P))

    nc = bass.Bass()
    f32 = mybir.dt.float32
    bf16 = mybir.dt.bfloat16
    # chunk-major DRAM layouts: one 2304B descriptor per (chunk, partition)
    Xd = nc.declare_dram_parameter("xin", [V // 2, 128, 1152], bf16, isOutput=False)
    Wd = nc.declare_dram_parameter("win", [U, 128, 1152], bf16, isOutput=False)
    Od = nc.declare_dram_parameter("o", [128, V * 64], bf16, isOutput=True)

    with TileContext(nc) as tc:
        with (
            tc.tile_pool(name="xp", bufs=4) as xp,
            tc.tile_pool(name="wp", bufs=4) as wp,
            tc.tile_pool(name="op", bufs=4) as op,
            tc.tile_pool(name="pp", bufs=8, space="PSUM") as pp,
        ):
            # one dedicated trigger queue per stream: a ring-full lock-wait on
            # one stream then never delays the other streams' triggers
            for gi, (j0, j1, v0, v1) in enumerate(groups):
                xt = xp.tile([128, (v1 - v0) * 576], bf16, tag="xt")
                nc.sync.dma_start(
                    out=xt, in_=Xd[v0 // 2 : v1 // 2].rearrange("c p e -> p c e")
                )
                wt = wp.tile([128, (j1 - j0) * 1152], bf16, tag="wt")
                nc.gpsimd.dma_start(
                    out=wt, in_=Wd[j0:j1].rearrange("j p e -> p j e")
                )
                ot = op.tile([128, (v1 - v0) * 64], bf16, tag="ot")
                for j in range(j0, j1):
                    pj = P[j]
                    xbase = (off[j] - v0) * 576  # job block: [KT, pj, 64]
                    ps = pp.tile([128, pj * 64], f32)
                    for k in range(KT):
                        nc.tensor.matmul(
                            ps,
                            wt[:, (j - j0) * 1152 + k * 128 : (j - j0) * 1152 + (k + 1) * 128],
                            xt[:, xbase + k * pj * 64 : xbase + (k + 1) * pj * 64],
                            start=(k == 0),
                            stop=(k == KT - 1),
                        )
                    obase = (off[j] - v0) * 64
                    nc.vector.tensor_copy(ot[:, obase : obase + pj * 64], ps)
                nc.scalar.dma_start(out=Od[:, v0 * 64 : v1 * 64], in_=ot)

    _split_excess_waits(nc)
    nc.finalize()
    _split_excess_waits(nc)
    return nc


def _split_excess_waits(nc, max_waits=1):
    # walrus codegen rejects instructions with >max sync waits; Tile's tail
    # drain can carry several. Hoist the excess onto preceding no-ops.
    import concourse.mybir as mybir

    for f in nc.m.functions:
        for b in f.blocks:
            i = 0
            while i < len(b.instructions):
                inst = b.instructions[i]
                si = inst.sync_info
                if si is not None and len(si.on_wait) > max_waits:
                    excess = list(si.on_wait[:-max_waits])
                    si.on_wait = list(si.on_wait[-max_waits:])
                    for w in excess:
                        nop = mybir.InstNoOp(
                            name=nc.get_next_instruction_name(),
                            engine=inst.engine,
                            sync_info=mybir.SyncInfo(on_wait=[w], on_update=[]),
                            bass_nofuse=True,
                        )
                        nc.register_instruction(nop)
                        b.instructions.insert(i, nop)
                        i += 1
                i += 1


def _prep_inputs(x, y, weight, cores, P):
    import ml_dtypes

    bf16 = ml_dtypes.bfloat16
    U = len(P)
    V = int(sum(P))
    off = np.concatenate([[0], np.cumsum(P)]).astype(int)

    x = np.ascontiguousarray(x, dtype=np.float32)
    weight = np.ascontiguousarray(weight, dtype=np.float32)
    # x[b, n, k*128+p] -> Xh[b, p, k, n]
    Xh = np.ascontiguousarray(x.reshape(B, N, KT, 128).transpose(0, 3, 2, 1)).astype(bf16)
    # weight[c, k*128+p, o] -> Wp[c, p, k, o] for the used classes only
    used = sorted({c for cr in cores for c, k in cr if c >= 0})
    cidx = {c: i for i, c in enumerate(used)}
    Wsel = weight[np.asarray(used, dtype=np.int64)]
    Wp = np.ascontiguousarray(
        Wsel.reshape(len(used), KT, 128, OUT_DIM).transpose(0, 2, 1, 3)
    ).reshape(len(used), 128, KT * OUT_DIM).astype(bf16)

    by_class = defaultdict(list)
    for i, c in enumerate(np.asarray(y).astype(np.int64)):
        by_class[int(c)].append(i)

    Xin = np.zeros((NCORES, 128, V * 576), dtype=bf16)
    Win = np.zeros((NCORES, U, 128, 1152), dtype=bf16)
    slotmap = np.full((NCORES, V), -1, dtype=np.int64)
    for m in range(NCORES):
        for j, (c, k) in enumerate(cores[m]):
            if k == 0:
                continue
            bs = by_class[c][:k]
            del by_class[c][:k]
            pj = P[j]
            # job block [128, KT, pj, 64]; real samples fill the first k lanes
            blk = Xin[m, :, off[j] * 576 : off[j + 1] * 576].reshape(128, KT, pj, 64)
            blk[:, :, :k, :] = Xh[bs].transpose(1, 2, 0, 3)
            Win[m, j] = Wp[cidx[c]]
            slotmap[m, off[j] : off[j] + k] = bs
    # chunk-major x: [V/2, 128, 1152elem]
    Xin = np.ascontiguousarray(Xin.reshape(NCORES, 128, V // 2, 1152).transpose(0, 2, 1, 3))
    return Xin, Win, slotmap, V


def kernel(x, y, weight, bias):
    from concourse.bass_utils import run_bass_kernel_spmd

    yi = np.asarray(y).astype(np.int64)
    cores, P = _schedule(yi)
    key = tuple(P)
    if _cache.get("key") != key:
        _cache["nc"] = _build_nc(P)
        _cache["key"] = key
    nc = _cache["nc"]

    Xin, Win, slotmap, V = _prep_inputs(x, y, weight, cores, P)
    in_maps = [{"xin": Xin[m], "win": Win[m]} for m in range(NCORES)]
    res = run_bass_kernel_spmd(nc, in_maps, list(range(NCORES)), **_cache.get("runkw", {}))
    _cache["last_result"] = res

    out = np.empty((B, N, OUT_DIM), dtype=np.float32)
    for m in range(NCORES):
        Oc = np.asarray(res.results[m]["o"], dtype=np.float32)
        Oc = Oc.reshape(128, V, 64).transpose(1, 2, 0)  # [slot, n, o]
        real = slotmap[m] >= 0
        out[slotmap[m][real]] = Oc[real]
    out += np.asarray(bias, dtype=np.float32)[yi][:, None, :]
    return out
